# revision 13
# baseline (speedup 1.0000x reference)
"""MoE-routed dynamic conv kernel for Trainium2 (8 NeuronCores, SPMD).

Problem: per-sample attention (global avg pool -> 1x1 conv -> sigmoid) mixes
K=4 expert 3x3 conv kernels; each sample is convolved with its own mixed
kernel.  x: (32, 256, 56, 56), att_w: (4, 256), weight: (4, 256, 256, 3, 3).

Strategy: data parallel over batch (4 samples per core, weights replicated).
x is zero-padded to (58, 58) on the host, so on device every conv tap
(kh, kw) is a flat contiguous slice of the padded image.  Per sample:
  - pooled sums via DVE free-dim reduce over the padded x tile
  - attention logits via tiny f32 PE matmuls against a host-side replicated
    att_w (gives att_k broadcast across all 128 partitions), sigmoid on ACT
  - expert mixing (agg = sum_k att_k * w_k) via 4 fused DVE ops per ci-block
  - conv as implicit GEMM in fp32r (FP22-truncated reads, full PE rate,
    even-count/aligned APs per the fp32r ISA restrictions): 18 matmuls
    (9 taps x 2 ci-blocks) accumulate into each PSUM chunk of 464 output
    columns (8 rows x 58); the two padded columns per row are discarded by
    the strided output DMA.

The per-sample stages are software-pipelined (att/mix of sample b+1 is
emitted before the conv of sample b) so the PE never waits on the
attention -> sigmoid -> mixing chain at sample boundaries.
"""

import sys

if "/opt/trn_rl_repo" not in sys.path:
    sys.path.insert(0, "/opt/trn_rl_repo")

import numpy as np

B_TOTAL = 32
N_CORES = 8
B_PER_CORE = B_TOTAL // N_CORES  # 4
CI = 256
CO = 256
K = 4
H = W = 56
PH = PW = 58
FLAT = PH * PW            # 3364 padded image
XT_F = FLAT + 4           # 3368: + tail pad for tap (2,2) overrun, host zeros
OUTF = H * PW             # 3248 output cols per co-block (56 rows x 58)
NCHUNK = 8 * PW           # 464 = 8 output rows per PSUM chunk (even, aligned)
NCHUNKS = 7               # 7 * 464 = 3248
TAPS = 9
TPC = TAPS * CO           # 2304 free elems per (k, ci-block) weight tile

_cache = {}


def _build_nc():
    from contextlib import ExitStack

    import concourse.bacc as bacc
    import concourse.mybir as mybir
    import concourse.tile as tile

    f32 = mybir.dt.float32
    f32r = mybir.dt.float32r
    AF = mybir.ActivationFunctionType
    ALU = mybir.AluOpType

    nc = bacc.Bacc("TRN2", target_bir_lowering=False, debug=False)
    x_p = nc.declare_dram_parameter("x", [B_PER_CORE, CI, XT_F], f32r, isOutput=False)
    w_p = nc.declare_dram_parameter("w", [K, CI, 3, 3, CO], f32, isOutput=False)
    ar_p = nc.declare_dram_parameter("attrep", [CI, K * 128], f32, isOutput=False)
    o_p = nc.declare_dram_parameter("out", [B_PER_CORE, CO, H, W], f32, isOutput=True)

    with ExitStack() as ctx:
        tc = ctx.enter_context(tile.TileContext(nc))
        pw = ctx.enter_context(tc.tile_pool(name="wpool", bufs=1))
        px = ctx.enter_context(tc.tile_pool(name="xpool", bufs=4))
        pagg = ctx.enter_context(tc.tile_pool(name="aggpool", bufs=4))
        pout = ctx.enter_context(tc.tile_pool(name="outpool", bufs=2))
        psml = ctx.enter_context(tc.tile_pool(name="small", bufs=4))
        pps = ctx.enter_context(tc.tile_pool(name="cpsum", bufs=6, space="PSUM"))
        ppsa = ctx.enter_context(tc.tile_pool(name="apsum", bufs=1, space="PSUM"))

        # Replicated attention weights (col j of block k = att_w[k, :]) and
        # the resident expert weights, free layout (k, tap, co) per ci-block.
        ar_sb = []
        for c in range(2):
            at = pw.tile([128, K * 128], f32, tag=f"ar{c}")
            nc.sync.dma_start(out=at[:, :], in_=ar_p[c * 128 : (c + 1) * 128, :])
            ar_sb.append(at)
        w_sb = []
        for c in range(2):
            wt = pw.tile([128, K * TPC], f32, tag=f"w{c}")
            for k in range(K):
                nc.sync.dma_start(
                    out=wt[:, k * TPC : (k + 1) * TPC],
                    in_=w_p[k, c * 128 : (c + 1) * 128].rearrange(
                        "ci kh kw co -> ci (kh kw co)"
                    ),
                )
            w_sb.append(wt)

        state = {}

        def stage_load(b):
            """Load padded x_b (both HWDGE engines) and pool."""
            xts = []
            pooleds = []
            for c in range(2):
                xt = px.tile([128, XT_F], f32r, tag="x")
                nc.scalar.dma_start(
                    out=xt[:, :], in_=x_p[b, c * 128 : (c + 1) * 128, :]
                )
                pl = psml.tile([128, 1], f32, tag="pooled")
                nc.vector.tensor_reduce(
                    pl[:, :], xt[:, 0:FLAT], axis=mybir.AxisListType.X, op=ALU.add
                )
                xts.append(xt)
                pooleds.append(pl)
            state[b] = (xts, pooleds)

        def stage_mix(b):
            """Attention, sigmoid, expert mixing for sample b."""
            xts, pooleds = state[b]
            # att[k] broadcast over all partitions: lhsT column j = att_w[k,:]
            # for every j, so out[j, 0] = dot(att_w[k], pooled) for all j.
            # Plain f32 matmuls (fp32r forbids odd moving counts like N=1).
            att_ps = ppsa.tile([128, K], f32, tag="attps")
            for k in range(K):
                for c in range(2):
                    nc.tensor.matmul(
                        att_ps[:, k : k + 1],
                        lhsT=ar_sb[c][:, k * 128 : (k + 1) * 128],
                        rhs=pooleds[c][:, :],
                        start=(c == 0),
                        stop=(c == 1),
                    )
            att_sb = psml.tile([128, K], f32, tag="attsb")
            nc.scalar.activation(
                att_sb[:, :], att_ps[:, :], AF.Sigmoid, scale=1.0 / (H * W)
            )

            # Expert mixing on DVE: agg = sum_k att_k * w_k, fused mul-add.
            aggs = []
            for c in range(2):
                ag = pagg.tile([128, TPC], f32r, tag="agg")
                nc.vector.tensor_scalar_mul(ag[:, :], w_sb[c][:, 0:TPC], att_sb[:, 0:1])
                for k in range(1, K):
                    nc.vector.scalar_tensor_tensor(
                        ag[:, :],
                        w_sb[c][:, k * TPC : (k + 1) * TPC],
                        att_sb[:, k : k + 1],
                        ag[:, :],
                        ALU.mult,
                        ALU.add,
                    )
                aggs.append(ag)
            state[b] = (xts, aggs)


        def stage_b(b):
            """Conv for sample b: per co-block, 7 PSUM chunks of 464 cols."""
            xts, aggs = state.pop(b)
            for cb in range(2):
                osb = pout.tile([128, OUTF], f32, tag="osb")
                for ch in range(NCHUNKS):
                    ps = pps.tile([128, NCHUNK], f32, tag="convps")
                    i = 0
                    for c in range(2):
                        for t in range(TAPS):
                            toff = (t // 3) * PW + (t % 3)
                            nc.tensor.matmul(
                                ps[:, :],
                                lhsT=aggs[c][
                                    :, t * CO + cb * 128 : t * CO + cb * 128 + 128
                                ],
                                rhs=xts[c][
                                    :, toff + ch * NCHUNK : toff + ch * NCHUNK + NCHUNK
                                ],
                                start=(i == 0),
                                stop=(i == 17),
                            )
                            i += 1
                    nc.scalar.copy(osb[:, ch * NCHUNK : (ch + 1) * NCHUNK], ps[:, :])
                # Strided DMA drops the two padded columns per row; split in
                # halves so the first transfer overlaps remaining evictions.
                o3 = osb[:, :].rearrange("p (h w) -> p h w", w=PW)
                half = H // 2
                for s in range(2):
                    nc.sync.dma_start(
                        out=o_p[b, cb * 128 : (cb + 1) * 128, s * half : (s + 1) * half],
                        in_=o3[:, s * half : (s + 1) * half, 0:W],
                    )

        # Software pipeline: loads lead their mix; att/mix of b+1 precedes
        # conv of b so the PE never stalls on the attention chain.
        stage_load(0)
        stage_mix(0)
        stage_load(1)
        stage_mix(1)
        stage_b(0)
        stage_load(2)
        stage_mix(2)
        stage_b(1)
        stage_load(3)
        stage_mix(3)
        stage_b(2)
        stage_b(3)

    nc.compile()
    return nc


def _get_nc():
    if "nc" not in _cache:
        _cache["nc"] = _build_nc()
    return _cache["nc"]


def _make_in_maps(x, att_w, weight):
    x = np.asarray(x, dtype=np.float32)
    att_w = np.asarray(att_w, dtype=np.float32)
    weight = np.asarray(weight, dtype=np.float32)
    # Host-side zero pad to (58, 58) + 4 tail elems, flattened per channel.
    xp = np.zeros((B_TOTAL, CI, XT_F), dtype=np.float32)
    xp[:, :, :FLAT] = np.pad(
        x, ((0, 0), (0, 0), (1, 1), (1, 1))
    ).reshape(B_TOTAL, CI, FLAT)
    # (K, Cout, Cin, kh, kw) -> (K, Cin, kh, kw, Cout) so the SBUF lhsT
    # layout [ci, (tap, co)] is a contiguous DMA.
    w_t = np.ascontiguousarray(weight.transpose(0, 2, 3, 4, 1))
    # (Cin, K*128): col j = att_w[j // 128, ci]
    att_rep = np.ascontiguousarray(np.repeat(att_w.T, 128, axis=1))
    return [
        {
            "x": np.ascontiguousarray(xp[i * B_PER_CORE : (i + 1) * B_PER_CORE]),
            "w": w_t,
            "attrep": att_rep,
        }
        for i in range(N_CORES)
    ]


def _run(x, att_w, weight, trace=False, **spmd_kwargs):
    from concourse.bass_utils import run_bass_kernel_spmd

    nc = _get_nc()
    in_maps = _make_in_maps(x, att_w, weight)
    res = run_bass_kernel_spmd(
        nc, in_maps, list(range(N_CORES)), trace=trace, **spmd_kwargs
    )
    out = np.concatenate([r["out"] for r in res.results], axis=0)
    return out.astype(np.float32, copy=False), res


def kernel(x, att_w, weight):
    out, _ = _run(x, att_w, weight)
    return out


# revision 14
# speedup vs baseline: 1.0356x; 1.0356x over previous
"""MoE-routed dynamic conv kernel for Trainium2 (8 NeuronCores, SPMD).

Problem: per-sample attention (global avg pool -> 1x1 conv -> sigmoid) mixes
K=4 expert 3x3 conv kernels; each sample is convolved with its own mixed
kernel.  x: (32, 256, 56, 56), att_w: (4, 256), weight: (4, 256, 256, 3, 3).

Strategy: data parallel over batch (4 samples per core, weights replicated).
x is zero-padded to (58, 58) on the host, so on device every conv tap
(kh, kw) is a flat contiguous slice of the padded image.  Per sample:
  - pooled sums via DVE free-dim reduce over the padded x tile
  - attention logits via tiny f32 PE matmuls against a host-side replicated
    att_w (gives att_k broadcast across all 128 partitions), sigmoid on ACT
  - expert mixing (agg = sum_k att_k * w_k) via 4 fused DVE ops per ci-block
  - conv as implicit GEMM in fp32r (FP22-truncated reads, full PE rate,
    even-count/aligned APs per the fp32r ISA restrictions): 18 matmuls
    (9 taps x 2 ci-blocks) accumulate into each PSUM chunk of 464 output
    columns (8 rows x 58); the two padded columns per row are discarded by
    the strided output DMA.

The per-sample stages are software-pipelined (att/mix of sample b+1 is
emitted before the conv of sample b) so the PE never waits on the
attention -> sigmoid -> mixing chain at sample boundaries.
"""

import sys

if "/opt/trn_rl_repo" not in sys.path:
    sys.path.insert(0, "/opt/trn_rl_repo")

import numpy as np

B_TOTAL = 32
N_CORES = 8
B_PER_CORE = B_TOTAL // N_CORES  # 4
CI = 256
CO = 256
K = 4
H = W = 56
PH = PW = 58
FLAT = PH * PW            # 3364 padded image
XT_F = FLAT + 4           # 3368: + tail pad for tap (2,2) overrun, host zeros
OUTF = H * W              # 3136 output cols per co-block (contiguous)
RPC = 8                   # output rows per PSUM chunk
NCHUNK = RPC * W          # 448 = 8 rows x 56 valid cols (even, aligned)
NCHUNKS = H // RPC        # 7
TAPS = 9
TPC = TAPS * CO           # 2304 free elems per (k, ci-block) weight tile

_cache = {}


def _build_nc():
    from contextlib import ExitStack

    import concourse.bacc as bacc
    import concourse.mybir as mybir
    import concourse.tile as tile

    f32 = mybir.dt.float32
    f32r = mybir.dt.float32r
    AF = mybir.ActivationFunctionType
    ALU = mybir.AluOpType

    nc = bacc.Bacc("TRN2", target_bir_lowering=False, debug=False)
    x_p = nc.declare_dram_parameter("x", [B_PER_CORE, CI, XT_F], f32r, isOutput=False)
    w_p = nc.declare_dram_parameter("w", [K, CI, 3, 3, CO], f32, isOutput=False)
    ar_p = nc.declare_dram_parameter("attrep", [CI, K * 128], f32, isOutput=False)
    o_p = nc.declare_dram_parameter("out", [B_PER_CORE, CO, H, W], f32, isOutput=True)

    with ExitStack() as ctx:
        tc = ctx.enter_context(tile.TileContext(nc))
        pw = ctx.enter_context(tc.tile_pool(name="wpool", bufs=1))
        px = ctx.enter_context(tc.tile_pool(name="xpool", bufs=4))
        pagg = ctx.enter_context(tc.tile_pool(name="aggpool", bufs=4))
        pout = ctx.enter_context(tc.tile_pool(name="outpool", bufs=2))
        psml = ctx.enter_context(tc.tile_pool(name="small", bufs=4))
        pps = ctx.enter_context(tc.tile_pool(name="cpsum", bufs=7, space="PSUM"))
        ppsa = ctx.enter_context(tc.tile_pool(name="apsum", bufs=1, space="PSUM"))

        # Replicated attention weights (col j of block k = att_w[k, :]) and
        # the resident expert weights, free layout (k, tap, co) per ci-block.
        ar_sb = []
        for c in range(2):
            at = pw.tile([128, K * 128], f32, tag=f"ar{c}")
            nc.sync.dma_start(out=at[:, :], in_=ar_p[c * 128 : (c + 1) * 128, :])
            ar_sb.append(at)
        w_sb = []
        for c in range(2):
            wt = pw.tile([128, K * TPC], f32, tag=f"w{c}")
            for k in range(K):
                nc.sync.dma_start(
                    out=wt[:, k * TPC : (k + 1) * TPC],
                    in_=w_p[k, c * 128 : (c + 1) * 128].rearrange(
                        "ci kh kw co -> ci (kh kw co)"
                    ),
                )
            w_sb.append(wt)

        state = {}

        def stage_load(b):
            """Load padded x_b (both HWDGE engines) and pool."""
            xts = []
            pooleds = []
            for c in range(2):
                xt = px.tile([128, XT_F], f32r, tag="x")
                nc.scalar.dma_start(
                    out=xt[:, :], in_=x_p[b, c * 128 : (c + 1) * 128, :]
                )
                pl = psml.tile([128, 1], f32, tag="pooled")
                nc.vector.tensor_reduce(
                    pl[:, :], xt[:, 0:FLAT], axis=mybir.AxisListType.X, op=ALU.add
                )
                xts.append(xt)
                pooleds.append(pl)
            state[b] = (xts, pooleds)

        def stage_mix(b):
            """Attention, sigmoid, expert mixing for sample b."""
            xts, pooleds = state[b]
            # att[k] broadcast over all partitions: lhsT column j = att_w[k,:]
            # for every j, so out[j, 0] = dot(att_w[k], pooled) for all j.
            # Plain f32 matmuls (fp32r forbids odd moving counts like N=1).
            att_ps = ppsa.tile([128, K], f32, tag="attps")
            for k in range(K):
                for c in range(2):
                    nc.tensor.matmul(
                        att_ps[:, k : k + 1],
                        lhsT=ar_sb[c][:, k * 128 : (k + 1) * 128],
                        rhs=pooleds[c][:, :],
                        start=(c == 0),
                        stop=(c == 1),
                    )
            att_sb = psml.tile([128, K], f32, tag="attsb")
            nc.scalar.activation(
                att_sb[:, :], att_ps[:, :], AF.Sigmoid, scale=1.0 / (H * W)
            )

            # Expert mixing on DVE: agg = sum_k att_k * w_k, fused mul-add.
            aggs = []
            for c in range(2):
                ag = pagg.tile([128, TPC], f32r, tag="agg")
                nc.vector.tensor_scalar_mul(ag[:, :], w_sb[c][:, 0:TPC], att_sb[:, 0:1])
                for k in range(1, K):
                    nc.vector.scalar_tensor_tensor(
                        ag[:, :],
                        w_sb[c][:, k * TPC : (k + 1) * TPC],
                        att_sb[:, k : k + 1],
                        ag[:, :],
                        ALU.mult,
                        ALU.add,
                    )
                aggs.append(ag)
            state[b] = (xts, aggs)


        def stage_b(b):
            """Conv for sample b: per co-block, 7 PSUM chunks of 8x56 cols.

            Each tap is a 2D window [8 rows, 56 valid cols] of the padded
            image (row stride 58), written to a dense [8, 56] PSUM chunk:
            no wasted pad columns, and the output buffer stays contiguous.
            """
            xts, aggs = state.pop(b)
            x3s = [
                xt[:, :FLAT].rearrange("p (h w) -> p h w", h=PH) for xt in xts
            ]
            for cb in range(2):
                osb = pout.tile([128, OUTF], f32, tag="osb")
                for ch in range(NCHUNKS):
                    r0 = ch * RPC
                    ps = pps.tile([128, NCHUNK], f32, tag="convps")
                    ps3 = ps[:, :].rearrange("p (h w) -> p h w", h=RPC)
                    i = 0
                    for c in range(2):
                        for t in range(TAPS):
                            dr, dc = t // 3, t % 3
                            nc.tensor.matmul(
                                ps3[:, :, :],
                                lhsT=aggs[c][
                                    :, t * CO + cb * 128 : t * CO + cb * 128 + 128
                                ],
                                rhs=x3s[c][
                                    :, r0 + dr : r0 + dr + RPC, dc : dc + W
                                ],
                                start=(i == 0),
                                stop=(i == 17),
                            )
                            i += 1
                    nc.scalar.copy(osb[:, ch * NCHUNK : (ch + 1) * NCHUNK], ps[:, :])
                # Contiguous output; split in halves so the first transfer
                # overlaps the remaining evictions.
                half = OUTF // 2
                for s in range(2):
                    nc.sync.dma_start(
                        out=o_p[b, cb * 128 : (cb + 1) * 128].rearrange(
                            "co h w -> co (h w)"
                        )[:, s * half : (s + 1) * half],
                        in_=osb[:, s * half : (s + 1) * half],
                    )

        # Software pipeline: loads lead their mix; att/mix of b+1 precedes
        # conv of b so the PE never stalls on the attention chain.
        stage_load(0)
        stage_mix(0)
        stage_load(1)
        stage_mix(1)
        stage_b(0)
        stage_load(2)
        stage_mix(2)
        stage_b(1)
        stage_load(3)
        stage_mix(3)
        stage_b(2)
        stage_b(3)

    nc.compile()
    return nc


def _get_nc():
    if "nc" not in _cache:
        _cache["nc"] = _build_nc()
    return _cache["nc"]


def _make_in_maps(x, att_w, weight):
    x = np.asarray(x, dtype=np.float32)
    att_w = np.asarray(att_w, dtype=np.float32)
    weight = np.asarray(weight, dtype=np.float32)
    # Host-side zero pad to (58, 58) + 4 tail elems, flattened per channel.
    xp = np.zeros((B_TOTAL, CI, XT_F), dtype=np.float32)
    xp[:, :, :FLAT] = np.pad(
        x, ((0, 0), (0, 0), (1, 1), (1, 1))
    ).reshape(B_TOTAL, CI, FLAT)
    # (K, Cout, Cin, kh, kw) -> (K, Cin, kh, kw, Cout) so the SBUF lhsT
    # layout [ci, (tap, co)] is a contiguous DMA.
    w_t = np.ascontiguousarray(weight.transpose(0, 2, 3, 4, 1))
    # (Cin, K*128): col j = att_w[j // 128, ci]
    att_rep = np.ascontiguousarray(np.repeat(att_w.T, 128, axis=1))
    return [
        {
            "x": np.ascontiguousarray(xp[i * B_PER_CORE : (i + 1) * B_PER_CORE]),
            "w": w_t,
            "attrep": att_rep,
        }
        for i in range(N_CORES)
    ]


def _run(x, att_w, weight, trace=False, **spmd_kwargs):
    from concourse.bass_utils import run_bass_kernel_spmd

    nc = _get_nc()
    in_maps = _make_in_maps(x, att_w, weight)
    res = run_bass_kernel_spmd(
        nc, in_maps, list(range(N_CORES)), trace=trace, **spmd_kwargs
    )
    out = np.concatenate([r["out"] for r in res.results], axis=0)
    return out.astype(np.float32, copy=False), res


def kernel(x, att_w, weight):
    out, _ = _run(x, att_w, weight)
    return out


# revision 19
# speedup vs baseline: 1.0570x; 1.0206x over previous
"""MoE-routed dynamic conv kernel for Trainium2 (8 NeuronCores, SPMD).

Problem: per-sample attention (global avg pool -> 1x1 conv -> sigmoid) mixes
K=4 expert 3x3 conv kernels; each sample is convolved with its own mixed
kernel.  x: (32, 256, 56, 56), att_w: (4, 256), weight: (4, 256, 256, 3, 3).

Strategy: data parallel over batch (4 samples per core, weights replicated).
x is zero-padded to (58, 58) on the host, so on device every conv tap
(kh, kw) is a flat contiguous slice of the padded image.  Per sample:
  - pooled sums via DVE free-dim reduce over the padded x tile
  - attention logits via tiny f32 PE matmuls against a host-side replicated
    att_w (gives att_k broadcast across all 128 partitions), sigmoid on ACT
  - expert mixing (agg = sum_k att_k * w_k) via 4 fused DVE ops per ci-block
  - conv as implicit GEMM in fp32r (FP22-truncated reads, full PE rate,
    even-count/aligned APs per the fp32r ISA restrictions): 18 matmuls
    (9 taps x 2 ci-blocks) accumulate into each PSUM chunk of 464 output
    columns (8 rows x 58); the two padded columns per row are discarded by
    the strided output DMA.

The per-sample stages are software-pipelined (att/mix of sample b+1 is
emitted before the conv of sample b) so the PE never waits on the
attention -> sigmoid -> mixing chain at sample boundaries.
"""

import sys

if "/opt/trn_rl_repo" not in sys.path:
    sys.path.insert(0, "/opt/trn_rl_repo")

import numpy as np

B_TOTAL = 32
N_CORES = 8
B_PER_CORE = B_TOTAL // N_CORES  # 4
CI = 256
CO = 256
K = 4
H = W = 56
PH = PW = 58
FLAT = PH * PW            # 3364 padded image
XT_F = FLAT + 4           # 3368: + tail pad for tap (2,2) overrun, host zeros
OUTF = H * W              # 3136 output cols per co-block (contiguous)
RPC = 8                   # output rows per PSUM chunk
NCHUNK = RPC * W          # 448 = 8 rows x 56 valid cols (even, aligned)
NCHUNKS = H // RPC        # 7
TAPS = 9
TPC = TAPS * CO           # 2304 free elems per (k, ci-block) weight tile

_cache = {}


def _build_nc():
    from contextlib import ExitStack

    import concourse.bacc as bacc
    import concourse.mybir as mybir
    import concourse.tile as tile

    f32 = mybir.dt.float32
    f32r = mybir.dt.float32r
    AF = mybir.ActivationFunctionType
    ALU = mybir.AluOpType

    nc = bacc.Bacc("TRN2", target_bir_lowering=False, debug=False)
    x_p = nc.declare_dram_parameter("x", [B_PER_CORE, CI, XT_F], f32r, isOutput=False)
    w_p = nc.declare_dram_parameter("w", [K, CI, 3, 3, CO], f32, isOutput=False)
    ar_p = nc.declare_dram_parameter("attrep", [CI, K * 128], f32, isOutput=False)
    o_p = nc.declare_dram_parameter("out", [B_PER_CORE, CO, H, W], f32, isOutput=True)

    with ExitStack() as ctx:
        tc = ctx.enter_context(tile.TileContext(nc))
        pw = ctx.enter_context(tc.tile_pool(name="wpool", bufs=1))
        px = ctx.enter_context(tc.tile_pool(name="xpool", bufs=4))
        pagg = ctx.enter_context(tc.tile_pool(name="aggpool", bufs=4))
        pout = ctx.enter_context(tc.tile_pool(name="outpool", bufs=2))
        psml = ctx.enter_context(tc.tile_pool(name="small", bufs=4))
        pps = ctx.enter_context(tc.tile_pool(name="cpsum", bufs=7, space="PSUM"))
        ppsa = ctx.enter_context(tc.tile_pool(name="apsum", bufs=1, space="PSUM"))

        # Replicated attention weights (col j of block k = att_w[k, :]) and
        # the resident expert weights, free layout (k, tap, co) per ci-block.
        ar_sb = []
        for c in range(2):
            at = pw.tile([128, K * 128], f32, tag=f"ar{c}")
            nc.sync.dma_start(out=at[:, :], in_=ar_p[c * 128 : (c + 1) * 128, :])
            ar_sb.append(at)
        w_sb = [
            pw.tile([128, K * TPC], f32, tag=f"w{c}", name=f"wt{c}")
            for c in range(2)
        ]
        for k in range(K):
            for c in range(2):
                nc.sync.dma_start(
                    out=w_sb[c][:, k * TPC : (k + 1) * TPC],
                    in_=w_p[k, c * 128 : (c + 1) * 128].rearrange(
                        "ci kh kw co -> ci (kh kw co)"
                    ),
                )

        state = {}

        def stage_load(b):
            """Load padded x_b (both HWDGE engines) and pool."""
            xts = []
            pooleds = []
            for c in range(2):
                xt = px.tile([128, XT_F], f32r, tag="x")
                eng = nc.scalar if c == 0 else nc.gpsimd
                eng.dma_start(out=xt[:, :], in_=x_p[b, c * 128 : (c + 1) * 128, :])
                pl = psml.tile([128, 1], f32, tag="pooled")
                nc.vector.tensor_reduce(
                    pl[:, :], xt[:, 0:FLAT], axis=mybir.AxisListType.X, op=ALU.add
                )
                xts.append(xt)
                pooleds.append(pl)
            state[b] = (xts, pooleds)

        def stage_att(b):
            """Attention matmuls + sigmoid for sample b."""
            xts, pooleds = state[b]
            # att[k] broadcast over all partitions: lhsT column j = att_w[k,:]
            # for every j, so out[j, 0] = dot(att_w[k], pooled) for all j.
            # Plain f32 matmuls (fp32r forbids odd moving counts like N=1).
            att_ps = ppsa.tile([128, K], f32, tag="attps")
            for k in range(K):
                for c in range(2):
                    nc.tensor.matmul(
                        att_ps[:, k : k + 1],
                        lhsT=ar_sb[c][:, k * 128 : (k + 1) * 128],
                        rhs=pooleds[c][:, :],
                        start=(c == 0),
                        stop=(c == 1),
                    )
            att_sb = psml.tile([128, K], f32, tag="attsb")
            nc.scalar.activation(
                att_sb[:, :], att_ps[:, :], AF.Sigmoid, scale=1.0 / (H * W)
            )
            state[b] = (xts, att_sb)

        def stage_mix(b):
            """Expert mixing on DVE: agg = sum_k att_k * w_k, fused mul-add."""
            xts, att_sb = state[b]
            aggs = []
            for c in range(2):
                ag = pagg.tile([128, TPC], f32r, tag="agg")
                nc.vector.tensor_scalar_mul(ag[:, :], w_sb[c][:, 0:TPC], att_sb[:, 0:1])
                for k in range(1, K):
                    nc.vector.scalar_tensor_tensor(
                        ag[:, :],
                        w_sb[c][:, k * TPC : (k + 1) * TPC],
                        att_sb[:, k : k + 1],
                        ag[:, :],
                        ALU.mult,
                        ALU.add,
                    )
                aggs.append(ag)
            state[b] = (xts, aggs)


        def stage_b(b, after_first_chunk=None):
            """Conv for sample b: per co-block, 7 PSUM chunks of 8x56 cols.

            Each tap is a 2D window [8 rows, 56 valid cols] of the padded
            image (row stride 58), written to a dense [8, 56] PSUM chunk:
            no wasted pad columns, and the output buffer stays contiguous.
            """
            xts, aggs = state.pop(b)
            x3s = [
                xt[:, :FLAT].rearrange("p (h w) -> p h w", h=PH) for xt in xts
            ]
            first_chunk_done = False
            for cb in range(2):
                osb = pout.tile([128, OUTF], f32, tag="osb")
                for ch in range(NCHUNKS):
                    r0 = ch * RPC
                    ps = pps.tile([128, NCHUNK], f32, tag="convps")
                    ps3 = ps[:, :].rearrange("p (h w) -> p h w", h=RPC)
                    i = 0
                    for c in range(2):
                        for t in range(TAPS):
                            dr, dc = t // 3, t % 3
                            nc.tensor.matmul(
                                ps3[:, :, :],
                                lhsT=aggs[c][
                                    :, t * CO + cb * 128 : t * CO + cb * 128 + 128
                                ],
                                rhs=x3s[c][
                                    :, r0 + dr : r0 + dr + RPC, dc : dc + W
                                ],
                                start=(i == 0),
                                stop=(i == 17),
                            )
                            i += 1
                    nc.scalar.copy(osb[:, ch * NCHUNK : (ch + 1) * NCHUNK], ps[:, :])
                    if not first_chunk_done:
                        first_chunk_done = True
                        if after_first_chunk is not None:
                            after_first_chunk()
                # Contiguous output; split in halves so the first transfer
                # overlaps the remaining evictions.
                half = OUTF // 2
                for s in range(2):
                    nc.sync.dma_start(
                        out=o_p[b, cb * 128 : (cb + 1) * 128].rearrange(
                            "co h w -> co (h w)"
                        )[:, s * half : (s + 1) * half],
                        in_=osb[:, s * half : (s + 1) * half],
                    )

        # Software pipeline: loads lead their mix; att/mix of b+1 precedes
        # conv of b so the PE never stalls on the attention chain.
        stage_load(0)
        stage_att(0)
        stage_mix(0)
        stage_load(1)

        def _att_mix_1():
            stage_att(1)
            stage_mix(1)

        stage_b(0, after_first_chunk=_att_mix_1)
        stage_load(2)
        stage_att(2)
        stage_mix(2)
        stage_b(1)
        stage_load(3)
        stage_att(3)
        stage_mix(3)
        stage_b(2)
        stage_b(3)

    nc.compile()
    return nc


def _get_nc():
    if "nc" not in _cache:
        _cache["nc"] = _build_nc()
    return _cache["nc"]


def _make_in_maps(x, att_w, weight):
    x = np.asarray(x, dtype=np.float32)
    att_w = np.asarray(att_w, dtype=np.float32)
    weight = np.asarray(weight, dtype=np.float32)
    # Host-side zero pad to (58, 58) + 4 tail elems, flattened per channel.
    xp = np.zeros((B_TOTAL, CI, XT_F), dtype=np.float32)
    xp[:, :, :FLAT] = np.pad(
        x, ((0, 0), (0, 0), (1, 1), (1, 1))
    ).reshape(B_TOTAL, CI, FLAT)
    # (K, Cout, Cin, kh, kw) -> (K, Cin, kh, kw, Cout) so the SBUF lhsT
    # layout [ci, (tap, co)] is a contiguous DMA.
    w_t = np.ascontiguousarray(weight.transpose(0, 2, 3, 4, 1))
    # (Cin, K*128): col j = att_w[j // 128, ci]
    att_rep = np.ascontiguousarray(np.repeat(att_w.T, 128, axis=1))
    return [
        {
            "x": np.ascontiguousarray(xp[i * B_PER_CORE : (i + 1) * B_PER_CORE]),
            "w": w_t,
            "attrep": att_rep,
        }
        for i in range(N_CORES)
    ]


def _run(x, att_w, weight, trace=False, **spmd_kwargs):
    from concourse.bass_utils import run_bass_kernel_spmd

    nc = _get_nc()
    in_maps = _make_in_maps(x, att_w, weight)
    res = run_bass_kernel_spmd(
        nc, in_maps, list(range(N_CORES)), trace=trace, **spmd_kwargs
    )
    out = np.concatenate([r["out"] for r in res.results], axis=0)
    return out.astype(np.float32, copy=False), res


def kernel(x, att_w, weight):
    out, _ = _run(x, att_w, weight)
    return out
